# revision 47
# baseline (speedup 1.0000x reference)
"""Distributed causal multi-head attention on one TRN2 chip (8 NeuronCores).

Problem: B=2, S=2048, D=1024, H=16, DH=64 (f32), causal softmax attention with
QKV + output projections.

Sharding (SPMD, one Bass graph for all 8 cores):
  core i -> batch b = i // 4, head group g = i % 4 (4 of 16 heads).
Each core projects Q/K/V for its 4 heads over the full sequence of its batch
and runs causal attention.  Per-head z (bf16) is AllGathered within each
batch's 4-core group one 512-row band at a time; each core then computes a
256-column slice of the output projection.  Core (b, g) returns
out[b, :, 256g:256g+256]; the host concatenates.

Schedule: a single software-pipelined band loop.  Attention of band t is
interleaved (at matmul-quantum granularity, via generators) with the QKV
projections of band t+1 and the output projection of band t-1, so the PE
queue never drains while ACT runs the exps.  The exp is batched over a
2-bank PSUM pair (both heads of a pair-packed group), diagonal chunks run
restricted q-ranges, and causal masking is a multiplicative bf16 0/1 mask
applied post-exp on the DVE.  Scores matmuls are row-tiled (contract=64,
two heads concurrently on PE row-groups 0-63/64-127).  The softmax
denominator rides a ones-column in V; its reciprocal uses the fast custom
DVE op and is broadcast across partitions on the idle GpSimd engine.
The final band's gather + output projection is split in two q-halves to
shorten the tail.
"""

import sys

for _p in ("/opt/trn_rl_repo", "/opt/pypackages"):
    if _p not in sys.path:
        sys.path.insert(0, _p)

from contextlib import ExitStack

import numpy as np

import concourse.bass as bass
import concourse.mybir as mybir
import concourse.tile as tile
from concourse import bacc
from concourse.bass_utils import run_bass_kernel_spmd

B, S, D, H, DH = 2, 2048, 1024, 16, 64
G = 4                       # heads per core
NCORES = 8
SCALE = float(np.sqrt(DH))
TQ = 512                    # query band (free dim)
NQT = S // TQ               # 4
KC = 128                    # key chunk (partition dim)
DC = 128                    # contraction d-chunk
NDC = D // DC               # 8
EG = G * DH                 # 256: packed head dim per group
VW = DH + 1                 # 65: head slot width in v_aug (ones column)
GW = G * VW                 # 260: v_aug width per k-chunk
DS = D // 4                 # 256: output D-column slice per core

F32 = mybir.dt.float32
BF16 = mybir.dt.bfloat16
EXP = mybir.ActivationFunctionType.Exp
LN = mybir.ActivationFunctionType.Ln
MULT = mybir.AluOpType.mult

GROUPS = [[0, 1, 2, 3], [4, 5, 6, 7]]

USE_GPSIMD_BCAST = False
USE_FAST_RECIP = False

_CACHE = {}


def _build() -> bass.Bass:
    nc = bacc.Bacc("TRN2", num_devices=NCORES, target_bir_lowering=False)

    xq = nc.declare_dram_parameter("xq", [D, S], BF16, isOutput=False)
    xk = nc.declare_dram_parameter("xk", [D, S], BF16, isOutput=False)
    xv = nc.declare_dram_parameter("xv", [D, S], BF16, isOutput=False)
    wq = nc.declare_dram_parameter("wq", [NDC, DC, EG], BF16, isOutput=False)
    wk = nc.declare_dram_parameter("wk", [NDC, DC, EG], BF16, isOutput=False)
    wv = nc.declare_dram_parameter("wv", [NDC, DC, EG], BF16, isOutput=False)
    wo = nc.declare_dram_parameter("wo", [NDC, DC, DS], BF16, isOutput=False)
    mask = nc.declare_dram_parameter("mask", [KC, G * TQ], BF16, isOutput=False)
    out_ext = nc.declare_dram_parameter("out", [S, DS], F32, isOutput=True)

    with ExitStack() as ctx:
        tc = ctx.enter_context(tile.TileContext(nc))
        const = ctx.enter_context(tc.tile_pool(name="const", bufs=1))
        dram = ctx.enter_context(tc.tile_pool(name="dram", bufs=1, space="DRAM"))
        xpool = ctx.enter_context(tc.tile_pool(name="x", bufs=2))
        epool = ctx.enter_context(tc.tile_pool(name="e", bufs=3))
        rpool = ctx.enter_context(tc.tile_pool(name="r", bufs=2))
        zgpool = ctx.enter_context(tc.tile_pool(name="zg", bufs=2))
        opool = ctx.enter_context(tc.tile_pool(name="o", bufs=2))
        # PSUM: pz 4 banks + sc pair 2 banks + proj 2 banks = 8
        pzpool = ctx.enter_context(tc.tile_pool(name="pz", bufs=1, space="PSUM"))
        scpool = ctx.enter_context(tc.tile_pool(name="sc", bufs=1, space="PSUM"))
        prpool = ctx.enter_context(tc.tile_pool(name="pr", bufs=2, space="PSUM"))

        # ---- persistent SBUF tiles ----
        wq_sb = const.tile([DC, NDC * EG], BF16, name="wq_sb")
        wk_sb = const.tile([DC, NDC * EG], BF16, name="wk_sb")
        wv_sb = const.tile([DC, NDC * EG], BF16, name="wv_sb")
        wo_sb = const.tile([DC, NDC * DS], BF16, name="wo_sb")
        mask_sb = const.tile([KC, G * TQ], BF16, name="mask_sb")
        vaug = const.tile([KC, (S // KC) * GW], BF16, name="vaug")
        q_sb = [const.tile([2 * DH, S], BF16, name=f"q_sb{p}") for p in range(2)]
        k_sb = [const.tile([2 * DH, S], BF16, name=f"k_sb{p}") for p in range(2)]
        z_sb = [const.tile([2 * DH, S], BF16, name=f"z_sb{p}") for p in range(2)]

        # ---- DRAM staging for the z AllGather ----
        # bands 0..2: full-band gathers; band 3: two q-halves to cut the tail
        zb = [dram.tile([EG, TQ], BF16, name=f"zb{t}") for t in range(NQT)]
        zg = [dram.tile([G * EG, TQ], BF16, name=f"zg{t}") for t in range(NQT)]
        dgi = dram.tile([KC, TQ], BF16, name="dgi")
        dgo = dram.tile([G * KC, TQ], BF16, name="dgo")

        # ---- prologue ----
        # dummy AllGather first: absorbs collective-stream init + rendezvous
        # cost so the first real gather is cheap
        dgz = const.tile([KC, TQ], BF16, name="dgz")
        nc.vector.memset(dgz[:], 0.0)
        ones_bf = const.tile([1, DH], BF16, name="ones_bf")
        nc.vector.memset(ones_bf[:], 1.0)
        nc.gpsimd.dma_start(dgi[:, :], dgz[:])
        nc.gpsimd.collective_compute(
            "AllGather",
            mybir.AluOpType.bypass,
            replica_groups=GROUPS,
            ins=[dgi.opt()],
            outs=[dgo.opt()],
        )
        # ones base for v_aug (value slots get overwritten by the v copies)
        nc.vector.memset(vaug[:], 1.0)

        x_t = {}

        def load_x(nm, src, t, eng=None, halves=1):
            xt = xpool.tile([DC, NDC * TQ], BF16, name=f"x{nm}", tag=f"x{nm}")
            src_v = src[:, :].rearrange("(c p) s -> p c s", p=DC)[
                :, :, t * TQ : (t + 1) * TQ
            ]
            dst_v = xt[:].rearrange("p (c s) -> p c s", c=NDC)
            hc = NDC // halves
            for i in range(halves):
                (eng or nc.sync).dma_start(
                    dst_v[:, i * hc : (i + 1) * hc, :],
                    src_v[:, i * hc : (i + 1) * hc, :],
                )
            x_t[(nm, t)] = xt

        # band-0 activations on the scalar hwdge queue so they stream in
        # parallel with the weight loads on the sync queue; first halves
        # first so the first projection matmuls start early
        load_x(
            "q", xq, 0, eng=nc.scalar, halves=2
        )
        for half in range(2):
            nc.sync.dma_start(
                wq_sb[:].rearrange("p (c e) -> p c e", c=NDC)[
                    :, half * 4 : (half + 1) * 4, :
                ],
                wq[:, :, :].rearrange("c p e -> p c e")[
                    :, half * 4 : (half + 1) * 4, :
                ],
            )
        load_x("k", xk, 0, eng=nc.sync)
        load_x("v", xv, 0, eng=nc.scalar)
        for wsrc, wdst in ((wk, wk_sb), (wv, wv_sb), (wo, wo_sb)):
            nc.sync.dma_start(
                wdst[:].rearrange("p (c e) -> p c e", c=NDC),
                wsrc[:, :, :].rearrange("c p e -> p c e"),
            )
        nc.sync.dma_start(mask_sb[:], mask[:, :])

        # ---- projections of one band (generators: yield between quanta) ----
        # Q must be ready before its band starts; K/V of band t are consumed
        # only by the band's last 4 kci iterations, so kvproj_gen(t) runs as
        # filler inside band t itself.
        def qproj_gen(t):
            if t > 0:
                load_x("q", xq, t)
                yield
            xt = x_t[("q", t)]
            for p in range(2):
                pq = prpool.tile([DC, TQ], F32, tag="pr", name="pq")
                for c in range(NDC):
                    nc.tensor.matmul(
                        pq[:],
                        wq_sb[:, c * EG + p * DC : c * EG + (p + 1) * DC],
                        xt[:, c * TQ : (c + 1) * TQ],
                        start=(c == 0),
                        stop=(c == NDC - 1),
                    )
                    if c % 4 == 3:
                        yield
                nc.vector.tensor_copy(q_sb[p][:, t * TQ : (t + 1) * TQ], pq[:])

        def kvproj_gen(t):
            if t > 0:
                load_x("k", xk, t)
                load_x("v", xv, t)
                yield
            xk_t, xv_t = x_t[("k", t)], x_t[("v", t)]
            for p in range(2):
                pq = prpool.tile([DC, TQ], F32, tag="pr", name="pk")
                for c in range(NDC):
                    nc.tensor.matmul(
                        pq[:],
                        wk_sb[:, c * EG + p * DC : c * EG + (p + 1) * DC],
                        xk_t[:, c * TQ : (c + 1) * TQ],
                        start=(c == 0),
                        stop=(c == NDC - 1),
                    )
                    if c % 4 == 3:
                        yield
                nc.vector.tensor_copy(k_sb[p][:, t * TQ : (t + 1) * TQ], pq[:])
            for sub in range(4):
                pv = prpool.tile([DC, TQ], F32, tag="pr", name="pv")
                for c in range(NDC):
                    nc.tensor.matmul(
                        pv[:, 0:EG],
                        xv_t[:, c * TQ + sub * KC : c * TQ + (sub + 1) * KC],
                        wv_sb[:, c * EG : (c + 1) * EG],
                        start=(c == 0),
                        stop=(c == NDC - 1),
                    )
                    if c % 4 == 3:
                        yield
                kci = t * 4 + sub
                base = kci * GW
                nc.vector.tensor_copy(
                    vaug[:, base : base + GW].rearrange(
                        "p (h w) -> p h w", h=G
                    )[:, :, 0:DH],
                    pv[:, 0:EG].rearrange("p (h e) -> p h e", h=G),
                )
            yield

        # ---- output projection of one band (generator) ----
        def oproj_band_gen(t, zgt, q0, nq):
            # out[t*TQ+q0 : +nq, :] = z_all^T @ W_O[:, cols of this core]
            nqs = nq // KC
            zg_sb = zgpool.tile([DC, NDC * nq], BF16, name="zg_sb", tag="zg")
            # two half-loads so the first matmuls can start on chunks 0-3
            # while chunks 4-7 stream in
            for ch in range(2):
                nc.gpsimd.dma_start(
                    zg_sb[:, ch * 4 * nq : (ch + 1) * 4 * nq].rearrange(
                        "p (c s) -> p c s", c=4
                    ),
                    zgt[:, :].rearrange("(c p) s -> p c s", p=DC)[
                        :, ch * 4 : (ch + 1) * 4, :
                    ],
                )
            yield
            o_sb = opool.tile([DC, nqs * DS], F32, name="o_sb", tag="o")
            for half in range(nqs // 2):
                po = prpool.tile([DC, 2 * DS], F32, tag="pr", name="po")
                for sub in range(2):
                    qs = half * 2 + sub
                    for c in range(NDC):
                        nc.tensor.matmul(
                            po[:, sub * DS : (sub + 1) * DS],
                            zg_sb[:, c * nq + qs * KC : c * nq + (qs + 1) * KC],
                            wo_sb[:, c * DS : (c + 1) * DS],
                            start=(c == 0),
                            stop=(c == NDC - 1),
                        )
                        if c % 4 == 3:
                            yield
                nc.vector.tensor_copy(
                    o_sb[:, half * 2 * DS : (half + 1) * 2 * DS], po[:]
                )
            nc.gpsimd.dma_start(
                out_ext[:, :].rearrange("(b p) d -> p b d", p=KC)[
                    :, 4 * t + q0 // KC : 4 * t + q0 // KC + nqs, :
                ],
                o_sb[:].rearrange("p (b d) -> p b d", b=nqs),
            )
            yield

        def pull(it, n):
            for _ in range(n):
                try:
                    next(it)
                except StopIteration:
                    return

        def drain(it):
            for _ in it:
                pass

        # ---- normalize + stage + gather a q-range of band t ----
        def normgather_gen(t, pz, zbt, zgt, last=False):
            # Free the pz banks ASAP: copy the unnormalized z out of PSUM
            # (the next band's first z-matmul only waits on these copies),
            # then build 1/den off to the side and normalize in-place in
            # SBUF.  The denominator lives on ONE partition (pz row 64),
            # where the DVE reciprocal runs single-lane (~6.7ns/elem); two
            # tiny SBUF<->SBUF DMAs redistribute it across 16 partitions
            # (128 elems/lane), and a third stride-0-source DMA broadcasts
            # the result across the 64 head-dim partitions.
            den_sb = rpool.tile([1, G * TQ], F32, tag="den_sb", name="den_sb")
            # DVE copy: the ACT queue is busy with the band's last exps
            nc.vector.tensor_copy(den_sb[:], pz[DH : DH + 1, :])
            for h in range(G):
                p_i, off = h // 2, (h % 2) * DH
                nc.vector.tensor_copy(
                    z_sb[p_i][off : off + DH, t * TQ : (t + 1) * TQ],
                    pz[0:DH, h * TQ : (h + 1) * TQ],
                )
            yield
            den_t = rpool.tile([16, KC], F32, tag="den_t", name="den_t")
            nc.sync.dma_start(den_t[:], den_sb[:])
            rec_t = rpool.tile([16, KC], BF16, tag="rec_t", name="rec_t")
            with nc.allow_low_precision(reason="softmax denom recip, bf16"):
                nc.vector.reciprocal(rec_t[:], den_t[:])
            den_r = rpool.tile([1, G * TQ], BF16, tag="denr", name="den_r")
            nc.sync.dma_start(den_r[:], rec_t[:])
            yield

            # rank-1 bf16 broadcast of 1/den on the PE, mul interleaved
            def bcmm(h):
                pb = prpool.tile([DC, TQ], F32, tag="pr", name="pb")
                nc.tensor.matmul(
                    pb[0:DH, :],
                    ones_bf[:],
                    den_r[:, h * TQ : (h + 1) * TQ],
                    start=True,
                    stop=True,
                )
                return pb

            def nmul(h, pb):
                p_i, off = h // 2, (h % 2) * DH
                zs = z_sb[p_i][off : off + DH, t * TQ : (t + 1) * TQ]
                nc.vector.tensor_mul(zs, zs, pb[0:DH, :])

            pbs = [bcmm(0), bcmm(1)]
            yield
            for h in range(G):
                nmul(h, pbs[h])
                if h + 2 < G:
                    pbs.append(bcmm(h + 2))
                yield
            for p in range(2):
                nc.gpsimd.dma_start(
                    zbt[p * KC : (p + 1) * KC, :],
                    z_sb[p][:, t * TQ : (t + 1) * TQ],
                )
            nc.gpsimd.collective_compute(
                "AllGather",
                mybir.AluOpType.bypass,
                replica_groups=GROUPS,
                ins=[zbt.opt()],
                outs=[zgt.opt()],
            )
            yield

        # ---- attention band with interleaved background work ----
        def attention_band(t, work, kv=None):
            nkc = 4 * (t + 1)
            pz = pzpool.tile([VW, G * TQ], F32, tag="pz", name="pz")
            for kci in range(nkc):
                if kv is not None and kci == 4 * t:
                    # this band's own k/v chunks start here: everything the
                    # kv generator hasn't emitted yet must land now
                    drain(kv)
                dc = kci - 4 * t
                qv = KC * dc if dc >= 0 else 0
                nq = TQ - qv
                es = []
                for pair in range(2):
                    sc = scpool.tile([KC, 2 * TQ], F32, tag="sc", name="sc")
                    for h2 in range(2):
                        nc.tensor.matmul(
                            sc[:, h2 * TQ + qv : (h2 + 1) * TQ],
                            k_sb[pair][
                                h2 * DH : (h2 + 1) * DH,
                                kci * KC : (kci + 1) * KC,
                            ],
                            q_sb[pair][
                                h2 * DH : (h2 + 1) * DH,
                                t * TQ + qv : (t + 1) * TQ,
                            ],
                            start=True,
                            stop=True,
                            tile_position=(h2 * DH, 0),
                        )
                    e = epool.tile([KC, 2 * TQ], BF16, tag=f"e{pair}", name="e")
                    sc_v = sc[:].rearrange("p (h q) -> p h q", h=2)[:, :, qv:TQ]
                    e_v = e[:].rearrange("p (h q) -> p h q", h=2)[:, :, qv:TQ]
                    nc.scalar.activation(e_v, sc_v, EXP)
                    if dc >= 0:
                        m_v = (
                            mask_sb[:, dc * TQ + qv : (dc + 1) * TQ]
                            .unsqueeze(1)
                            .broadcast_to([KC, 2, nq])
                        )
                        nc.vector.tensor_mul(e_v, e_v, m_v)
                    es.append(e)
                    pull(work, 2 if pair == 0 else 1)
                for h in range(G):
                    e = es[h // 2]
                    h2 = h % 2
                    nc.tensor.matmul(
                        pz[:, h * TQ + qv : (h + 1) * TQ],
                        vaug[:, kci * GW + h * VW : kci * GW + (h + 1) * VW],
                        e[:, h2 * TQ + qv : (h2 + 1) * TQ],
                        start=(kci == 0),
                        stop=(kci == nkc - 1),
                    )
                pull(work, 1)
            return pz

        # ---- the pipeline ----
        # attention(t) runs with the previous band's normalize+gather, the
        # next band's projections, and the t-1 output projection interleaved
        # into its instruction stream
        drain(qproj_gen(0))
        drain(kvproj_gen(0))
        ogens = {}
        pend = None
        for t in range(NQT):
            gens = []
            # ng first: its pz-freeing copies must precede this band's first
            # z-matmul in program order (pz pool bufs=1)
            if pend is not None:
                gens.append(pend)
            kv = None
            if t > 0:
                # this band's own K/V: consumed only by its last 4 kci
                kv = kvproj_gen(t)
                gens.append(kv)
            if t + 1 < NQT:
                gens.append(qproj_gen(t + 1))
            # output projections are deferred one extra band (t-1's oproj in
            # band t+1) so the first one lands well past the runtime's
            # one-time collective barrier
            if t == 2:
                gens.append(ogens[0])
            elif t == 3:
                gens += [ogens[1], ogens[2]]
            work = _chain(gens)
            pz = attention_band(t, work, kv=kv)
            drain(work)
            if t < NQT - 1:
                pend = normgather_gen(t, pz, zb[t], zg[t])
                ogens[t] = oproj_band_gen(t, zg[t], 0, TQ)
            else:
                drain(normgather_gen(t, pz, zb[t], zg[t], last=True))
                drain(oproj_band_gen(t, zg[t], 0, TQ))

    nc.compile()
    return nc


def _chain(gens):
    for g in gens:
        yield from g


def _get_graph() -> bass.Bass:
    if "nc" not in _CACHE:
        _CACHE["nc"] = _build()
    return _CACHE["nc"]


def _make_mask() -> np.ndarray:
    import ml_dtypes

    m = np.empty((KC, G * TQ), np.float32)
    x = np.arange(KC)[:, None]
    y = np.arange(TQ)[None, :]
    for dc in range(G):
        m[:, dc * TQ : (dc + 1) * TQ] = (dc * KC + x <= y).astype(np.float32)
    return m.astype(ml_dtypes.bfloat16)


def _make_in_maps(inputs: dict) -> list[dict]:
    import ml_dtypes

    bf16 = ml_dtypes.bfloat16
    qx = np.asarray(inputs["query_input"], np.float32).astype(bf16)
    kx = np.asarray(inputs["key_input"], np.float32).astype(bf16)
    vx = np.asarray(inputs["value_input"], np.float32).astype(bf16)
    # fold the attention scale into W_Q on the host
    WQ = (np.asarray(inputs["W_Q"], np.float32) / SCALE).astype(bf16)
    WK = np.asarray(inputs["W_K"], np.float32).astype(bf16)
    WV = np.asarray(inputs["W_V"], np.float32).astype(bf16)
    WO = np.asarray(inputs["W_O"], np.float32).astype(bf16)

    mask = _make_mask()
    xT = {
        (nm, b): np.ascontiguousarray(arr[b].T)
        for nm, arr in (("xq", qx), ("xk", kx), ("xv", vx))
        for b in range(B)
    }
    WO_flat = WO.reshape(H * DH, D)  # e' = h*64 + e, h-major (AllGather order)
    wmaps = []
    for g in range(G):
        hs = slice(g * G, (g + 1) * G)

        def prep(w):
            return np.ascontiguousarray(
                w[hs].transpose(1, 0, 2).reshape(D, EG).reshape(NDC, DC, EG)
            )

        wmaps.append(
            {
                "wq": prep(WQ),
                "wk": prep(WK),
                "wv": prep(WV),
                "wo": np.ascontiguousarray(
                    WO_flat[:, g * DS : (g + 1) * DS].reshape(NDC, DC, DS)
                ),
            }
        )

    in_maps = []
    for core in range(NCORES):
        b, g = core // G, core % G
        m = {
            "xq": xT[("xq", b)],
            "xk": xT[("xk", b)],
            "xv": xT[("xv", b)],
            "mask": mask,
        }
        m.update(wmaps[g])
        in_maps.append(m)
    return in_maps


def _assemble(results: list[dict]) -> np.ndarray:
    out = np.empty((B, S, D), np.float32)
    for core in range(NCORES):
        b, g = core // G, core % G
        out[b, :, g * DS : (g + 1) * DS] = results[core]["out"]
    return out


def run(inputs: dict, trace: bool = False):
    """Run on hardware; returns (output, BassKernelResults)."""
    nc = _get_graph()
    res = run_bass_kernel_spmd(
        nc, _make_in_maps(inputs), core_ids=list(range(NCORES)), trace=trace
    )
    return _assemble(res.results), res


def kernel(**inputs) -> np.ndarray:
    out, _ = run(inputs)
    return out


# revision 49
# speedup vs baseline: 1.0110x; 1.0110x over previous
"""Distributed causal multi-head attention on one TRN2 chip (8 NeuronCores).

Problem: B=2, S=2048, D=1024, H=16, DH=64 (f32), causal softmax attention with
QKV + output projections.

Sharding (SPMD, one Bass graph for all 8 cores):
  core i -> batch b = i // 4, head group g = i % 4 (4 of 16 heads).
Each core projects Q/K/V for its 4 heads over the full sequence of its batch
and runs causal attention.  Per-head z (bf16) is AllGathered within each
batch's 4-core group one 512-row band at a time; each core then computes a
256-column slice of the output projection.  Core (b, g) returns
out[b, :, 256g:256g+256]; the host concatenates.

Schedule: a single software-pipelined band loop.  Attention of band t is
interleaved (at matmul-quantum granularity, via generators) with the QKV
projections of band t+1 and the output projection of band t-1, so the PE
queue never drains while ACT runs the exps.  The exp is batched over a
2-bank PSUM pair (both heads of a pair-packed group), diagonal chunks run
restricted q-ranges, and causal masking is a multiplicative bf16 0/1 mask
applied post-exp on the DVE.  Scores matmuls are row-tiled (contract=64,
two heads concurrently on PE row-groups 0-63/64-127).  The softmax
denominator rides a ones-column in V; its reciprocal uses the fast custom
DVE op and is broadcast across partitions on the idle GpSimd engine.
The final band's gather + output projection is split in two q-halves to
shorten the tail.
"""

import sys

for _p in ("/opt/trn_rl_repo", "/opt/pypackages"):
    if _p not in sys.path:
        sys.path.insert(0, _p)

from contextlib import ExitStack

import numpy as np

import concourse.bass as bass
import concourse.mybir as mybir
import concourse.tile as tile
from concourse import bacc
from concourse.bass_utils import run_bass_kernel_spmd

B, S, D, H, DH = 2, 2048, 1024, 16, 64
G = 4                       # heads per core
NCORES = 8
SCALE = float(np.sqrt(DH))
TQ = 512                    # query band (free dim)
NQT = S // TQ               # 4
KC = 128                    # key chunk (partition dim)
DC = 128                    # contraction d-chunk
NDC = D // DC               # 8
EG = G * DH                 # 256: packed head dim per group
VW = DH + 1                 # 65: head slot width in v_aug (ones column)
GW = G * VW                 # 260: v_aug width per k-chunk
DS = D // 4                 # 256: output D-column slice per core

F32 = mybir.dt.float32
BF16 = mybir.dt.bfloat16
EXP = mybir.ActivationFunctionType.Exp
LN = mybir.ActivationFunctionType.Ln
MULT = mybir.AluOpType.mult

GROUPS = [[0, 1, 2, 3], [4, 5, 6, 7]]

USE_GPSIMD_BCAST = False
USE_FAST_RECIP = False

_CACHE = {}


def _build() -> bass.Bass:
    nc = bacc.Bacc("TRN2", num_devices=NCORES, target_bir_lowering=False)

    xq = nc.declare_dram_parameter("xq", [D, S], BF16, isOutput=False)
    xk = nc.declare_dram_parameter("xk", [D, S], BF16, isOutput=False)
    xv = nc.declare_dram_parameter("xv", [D, S], BF16, isOutput=False)
    wq = nc.declare_dram_parameter("wq", [NDC, DC, EG], BF16, isOutput=False)
    wk = nc.declare_dram_parameter("wk", [NDC, DC, EG], BF16, isOutput=False)
    wv = nc.declare_dram_parameter("wv", [NDC, DC, EG], BF16, isOutput=False)
    wo = nc.declare_dram_parameter("wo", [NDC, DC, DS], BF16, isOutput=False)
    mask = nc.declare_dram_parameter("mask", [KC, G * TQ], BF16, isOutput=False)
    out_ext = nc.declare_dram_parameter("out", [S, DS], F32, isOutput=True)

    with ExitStack() as ctx:
        tc = ctx.enter_context(tile.TileContext(nc))
        const = ctx.enter_context(tc.tile_pool(name="const", bufs=1))
        dram = ctx.enter_context(tc.tile_pool(name="dram", bufs=1, space="DRAM"))
        xpool = ctx.enter_context(tc.tile_pool(name="x", bufs=2))
        epool = ctx.enter_context(tc.tile_pool(name="e", bufs=3))
        rpool = ctx.enter_context(tc.tile_pool(name="r", bufs=2))
        zgpool = ctx.enter_context(tc.tile_pool(name="zg", bufs=2))
        opool = ctx.enter_context(tc.tile_pool(name="o", bufs=2))
        # PSUM: pz 4 banks + sc pair 2 banks + proj 2 banks = 8
        pzpool = ctx.enter_context(tc.tile_pool(name="pz", bufs=1, space="PSUM"))
        scpool = ctx.enter_context(tc.tile_pool(name="sc", bufs=1, space="PSUM"))
        prpool = ctx.enter_context(tc.tile_pool(name="pr", bufs=2, space="PSUM"))

        # ---- persistent SBUF tiles ----
        wq_sb = const.tile([DC, NDC * EG], BF16, name="wq_sb")
        wk_sb = const.tile([DC, NDC * EG], BF16, name="wk_sb")
        wv_sb = const.tile([DC, NDC * EG], BF16, name="wv_sb")
        wo_sb = const.tile([DC, NDC * DS], BF16, name="wo_sb")
        mask_sb = const.tile([KC, G * TQ], BF16, name="mask_sb")
        vaug = const.tile([KC, (S // KC) * GW], BF16, name="vaug")
        q_sb = [const.tile([2 * DH, S], BF16, name=f"q_sb{p}") for p in range(2)]
        k_sb = [const.tile([2 * DH, S], BF16, name=f"k_sb{p}") for p in range(2)]
        z_sb = [const.tile([2 * DH, S], BF16, name=f"z_sb{p}") for p in range(2)]

        # ---- DRAM staging for the z AllGather ----
        # bands 0..2: full-band gathers; band 3: two q-halves to cut the tail
        zb = [dram.tile([EG, TQ], BF16, name=f"zb{t}") for t in range(NQT)]
        zg = [dram.tile([G * EG, TQ], BF16, name=f"zg{t}") for t in range(NQT)]
        dgi = dram.tile([KC, TQ], BF16, name="dgi")
        dgo = dram.tile([G * KC, TQ], BF16, name="dgo")

        # ---- prologue ----
        # dummy AllGather first: absorbs collective-stream init + rendezvous
        # cost so the first real gather is cheap
        dgz = const.tile([KC, TQ], BF16, name="dgz")
        nc.vector.memset(dgz[:], 0.0)
        ones_bf = const.tile([1, DH], BF16, name="ones_bf")
        nc.vector.memset(ones_bf[:], 1.0)
        nc.gpsimd.dma_start(dgi[:, :], dgz[:])
        nc.gpsimd.collective_compute(
            "AllGather",
            mybir.AluOpType.bypass,
            replica_groups=GROUPS,
            ins=[dgi.opt()],
            outs=[dgo.opt()],
        )
        # ones base for v_aug (value slots get overwritten by the v copies)
        nc.vector.memset(vaug[:], 1.0)

        x_t = {}

        def load_x(nm, src, t, eng=None, halves=1):
            xt = xpool.tile([DC, NDC * TQ], BF16, name=f"x{nm}", tag=f"x{nm}")
            src_v = src[:, :].rearrange("(c p) s -> p c s", p=DC)[
                :, :, t * TQ : (t + 1) * TQ
            ]
            dst_v = xt[:].rearrange("p (c s) -> p c s", c=NDC)
            hc = NDC // halves
            for i in range(halves):
                (eng or nc.sync).dma_start(
                    dst_v[:, i * hc : (i + 1) * hc, :],
                    src_v[:, i * hc : (i + 1) * hc, :],
                )
            x_t[(nm, t)] = xt

        # band-0 activations on the scalar hwdge queue so they stream in
        # parallel with the weight loads on the sync queue; first halves
        # first so the first projection matmuls start early
        load_x(
            "q", xq, 0, eng=nc.scalar, halves=2
        )
        for half in range(2):
            nc.sync.dma_start(
                wq_sb[:].rearrange("p (c e) -> p c e", c=NDC)[
                    :, half * 4 : (half + 1) * 4, :
                ],
                wq[:, :, :].rearrange("c p e -> p c e")[
                    :, half * 4 : (half + 1) * 4, :
                ],
            )
        load_x("k", xk, 0, eng=nc.sync)
        load_x("v", xv, 0, eng=nc.scalar)
        for wsrc, wdst in ((wk, wk_sb), (wv, wv_sb), (wo, wo_sb)):
            nc.sync.dma_start(
                wdst[:].rearrange("p (c e) -> p c e", c=NDC),
                wsrc[:, :, :].rearrange("c p e -> p c e"),
            )
        nc.sync.dma_start(mask_sb[:], mask[:, :])

        # ---- projections of one band (generators: yield between quanta) ----
        # Q must be ready before its band starts; K/V of band t are consumed
        # only by the band's last 4 kci iterations, so kvproj_gen(t) runs as
        # filler inside band t itself.
        def qproj_gen(t):
            if t > 0:
                load_x("q", xq, t)
                yield
            xt = x_t[("q", t)]
            for p in range(2):
                pq = prpool.tile([DC, TQ], F32, tag="pr", name="pq")
                for c in range(NDC):
                    nc.tensor.matmul(
                        pq[:],
                        wq_sb[:, c * EG + p * DC : c * EG + (p + 1) * DC],
                        xt[:, c * TQ : (c + 1) * TQ],
                        start=(c == 0),
                        stop=(c == NDC - 1),
                    )
                    if c % 4 == 3:
                        yield
                nc.vector.tensor_copy(q_sb[p][:, t * TQ : (t + 1) * TQ], pq[:])

        def kvproj_gen(t):
            if t > 0:
                load_x("k", xk, t)
                load_x("v", xv, t)
                yield
            xk_t, xv_t = x_t[("k", t)], x_t[("v", t)]
            for p in range(2):
                pq = prpool.tile([DC, TQ], F32, tag="pr", name="pk")
                for c in range(NDC):
                    nc.tensor.matmul(
                        pq[:],
                        wk_sb[:, c * EG + p * DC : c * EG + (p + 1) * DC],
                        xk_t[:, c * TQ : (c + 1) * TQ],
                        start=(c == 0),
                        stop=(c == NDC - 1),
                    )
                    if c % 4 == 3:
                        yield
                nc.vector.tensor_copy(k_sb[p][:, t * TQ : (t + 1) * TQ], pq[:])
            for sub in range(4):
                pv = prpool.tile([DC, TQ], F32, tag="pr", name="pv")
                for c in range(NDC):
                    nc.tensor.matmul(
                        pv[:, 0:EG],
                        xv_t[:, c * TQ + sub * KC : c * TQ + (sub + 1) * KC],
                        wv_sb[:, c * EG : (c + 1) * EG],
                        start=(c == 0),
                        stop=(c == NDC - 1),
                    )
                    if c % 4 == 3:
                        yield
                kci = t * 4 + sub
                base = kci * GW
                nc.vector.tensor_copy(
                    vaug[:, base : base + GW].rearrange(
                        "p (h w) -> p h w", h=G
                    )[:, :, 0:DH],
                    pv[:, 0:EG].rearrange("p (h e) -> p h e", h=G),
                )
            yield

        # ---- output projection of one band (generator) ----
        def oproj_band_gen(t, zgt, q0, nq):
            # out[t*TQ+q0 : +nq, :] = z_all^T @ W_O[:, cols of this core]
            nqs = nq // KC
            zg_sb = zgpool.tile([DC, NDC * nq], BF16, name="zg_sb", tag="zg")
            # two half-loads so the first matmuls can start on chunks 0-3
            # while chunks 4-7 stream in
            for ch in range(2):
                nc.gpsimd.dma_start(
                    zg_sb[:, ch * 4 * nq : (ch + 1) * 4 * nq].rearrange(
                        "p (c s) -> p c s", c=4
                    ),
                    zgt[:, :].rearrange("(c p) s -> p c s", p=DC)[
                        :, ch * 4 : (ch + 1) * 4, :
                    ],
                )
            yield
            o_sb = opool.tile([DC, nqs * DS], F32, name="o_sb", tag="o")
            for half in range(nqs // 2):
                po = prpool.tile([DC, 2 * DS], F32, tag="pr", name="po")
                for sub in range(2):
                    qs = half * 2 + sub
                    for c in range(NDC):
                        nc.tensor.matmul(
                            po[:, sub * DS : (sub + 1) * DS],
                            zg_sb[:, c * nq + qs * KC : c * nq + (qs + 1) * KC],
                            wo_sb[:, c * DS : (c + 1) * DS],
                            start=(c == 0),
                            stop=(c == NDC - 1),
                        )
                        if c % 4 == 3:
                            yield
                nc.vector.tensor_copy(
                    o_sb[:, half * 2 * DS : (half + 1) * 2 * DS], po[:]
                )
            nc.gpsimd.dma_start(
                out_ext[:, :].rearrange("(b p) d -> p b d", p=KC)[
                    :, 4 * t + q0 // KC : 4 * t + q0 // KC + nqs, :
                ],
                o_sb[:].rearrange("p (b d) -> p b d", b=nqs),
            )
            yield

        def pull(it, n):
            for _ in range(n):
                try:
                    next(it)
                except StopIteration:
                    return

        def drain(it):
            for _ in it:
                pass

        # ---- normalize + stage + gather a q-range of band t ----
        def normgather_gen(t, pz, zbt, zgt, last=False):
            # Free the pz banks ASAP: copy the unnormalized z out of PSUM
            # (the next band's first z-matmul only waits on these copies),
            # then build 1/den off to the side and normalize in-place in
            # SBUF.  The denominator lives on ONE partition (pz row 64),
            # where the DVE reciprocal runs single-lane (~6.7ns/elem); two
            # tiny SBUF<->SBUF DMAs redistribute it across 16 partitions
            # (128 elems/lane), and a third stride-0-source DMA broadcasts
            # the result across the 64 head-dim partitions.
            den_sb = rpool.tile([1, G * TQ], F32, tag="den_sb", name="den_sb")
            # DVE copy: the ACT queue is busy with the band's last exps
            nc.vector.tensor_copy(den_sb[:], pz[DH : DH + 1, :])
            for h in range(G):
                p_i, off = h // 2, (h % 2) * DH
                nc.vector.tensor_copy(
                    z_sb[p_i][off : off + DH, t * TQ : (t + 1) * TQ],
                    pz[0:DH, h * TQ : (h + 1) * TQ],
                )
            yield
            den_t = rpool.tile([16, KC], F32, tag="den_t", name="den_t")
            nc.sync.dma_start(den_t[:], den_sb[:])
            rec_t = rpool.tile([16, KC], BF16, tag="rec_t", name="rec_t")
            with nc.allow_low_precision(reason="softmax denom recip, bf16"):
                nc.vector.reciprocal(rec_t[:], den_t[:])
            den_r = rpool.tile([1, G * TQ], BF16, tag="denr", name="den_r")
            nc.sync.dma_start(den_r[:], rec_t[:])
            yield

            # rank-1 bf16 broadcast of 1/den on the PE, mul interleaved
            def bcmm(h):
                pb = prpool.tile([DC, TQ], F32, tag="pr", name="pb")
                nc.tensor.matmul(
                    pb[0:DH, :],
                    ones_bf[:],
                    den_r[:, h * TQ : (h + 1) * TQ],
                    start=True,
                    stop=True,
                )
                return pb

            def nmul(h, pb):
                p_i, off = h // 2, (h % 2) * DH
                zs = z_sb[p_i][off : off + DH, t * TQ : (t + 1) * TQ]
                nc.vector.tensor_mul(zs, zs, pb[0:DH, :])

            pbs = [bcmm(0), bcmm(1)]
            yield
            for h in range(G):
                nmul(h, pbs[h])
                if h + 2 < G:
                    pbs.append(bcmm(h + 2))
                yield
            for p in range(2):
                nc.gpsimd.dma_start(
                    zbt[p * KC : (p + 1) * KC, :],
                    z_sb[p][:, t * TQ : (t + 1) * TQ],
                )
            nc.gpsimd.collective_compute(
                "AllGather",
                mybir.AluOpType.bypass,
                replica_groups=GROUPS,
                ins=[zbt.opt()],
                outs=[zgt.opt()],
            )
            yield

        # ---- attention band with interleaved background work ----
        def attention_band(t, work, kv=None):
            nkc = 4 * (t + 1)
            pz = pzpool.tile([VW, G * TQ], F32, tag="pz", name="pz")
            for kci in range(nkc):
                if kv is not None:
                    if t == 0:
                        # band 0 consumes its own k/v immediately: k copies
                        # land by quantum ~7, v-sub kci by quantum ~10+3*kci
                        pull(kv, (10, 3, 3, 99)[kci] if kci < 4 else 0)
                    elif kci == 4 * t:
                        # this band's own k/v chunks start here: everything
                        # the kv generator hasn't emitted yet must land now
                        drain(kv)
                dc = kci - 4 * t
                qv = KC * dc if dc >= 0 else 0
                nq = TQ - qv
                es = []
                for pair in range(2):
                    sc = scpool.tile([KC, 2 * TQ], F32, tag="sc", name="sc")
                    for h2 in range(2):
                        nc.tensor.matmul(
                            sc[:, h2 * TQ + qv : (h2 + 1) * TQ],
                            k_sb[pair][
                                h2 * DH : (h2 + 1) * DH,
                                kci * KC : (kci + 1) * KC,
                            ],
                            q_sb[pair][
                                h2 * DH : (h2 + 1) * DH,
                                t * TQ + qv : (t + 1) * TQ,
                            ],
                            start=True,
                            stop=True,
                            tile_position=(h2 * DH, 0),
                        )
                    e = epool.tile([KC, 2 * TQ], BF16, tag=f"e{pair}", name="e")
                    sc_v = sc[:].rearrange("p (h q) -> p h q", h=2)[:, :, qv:TQ]
                    e_v = e[:].rearrange("p (h q) -> p h q", h=2)[:, :, qv:TQ]
                    nc.scalar.activation(e_v, sc_v, EXP)
                    if dc >= 0:
                        m_v = (
                            mask_sb[:, dc * TQ + qv : (dc + 1) * TQ]
                            .unsqueeze(1)
                            .broadcast_to([KC, 2, nq])
                        )
                        nc.vector.tensor_mul(e_v, e_v, m_v)
                    es.append(e)
                    pull(work, 2 if pair == 0 else 1)
                for h in range(G):
                    e = es[h // 2]
                    h2 = h % 2
                    nc.tensor.matmul(
                        pz[:, h * TQ + qv : (h + 1) * TQ],
                        vaug[:, kci * GW + h * VW : kci * GW + (h + 1) * VW],
                        e[:, h2 * TQ + qv : (h2 + 1) * TQ],
                        start=(kci == 0),
                        stop=(kci == nkc - 1),
                    )
                pull(work, 1)
            return pz

        # ---- the pipeline ----
        # attention(t) runs with the previous band's normalize+gather, the
        # next band's projections, and the t-1 output projection interleaved
        # into its instruction stream
        drain(qproj_gen(0))
        ogens = {}
        pend = None
        for t in range(NQT):
            gens = []
            # ng first: its pz-freeing copies must precede this band's first
            # z-matmul in program order (pz pool bufs=1)
            if pend is not None:
                gens.append(pend)
            # this band's own K/V: consumed only by its last 4 kci (band 0:
            # streamed just-in-time via explicit pulls in attention_band)
            kv = kvproj_gen(t)
            gens.append(kv)
            if t + 1 < NQT:
                gens.append(qproj_gen(t + 1))
            # output projections are deferred one extra band (t-1's oproj in
            # band t+1) so the first one lands well past the runtime's
            # one-time collective barrier
            if t == 2:
                gens.append(ogens[0])
            elif t == 3:
                gens += [ogens[1], ogens[2]]
            work = _chain(gens)
            pz = attention_band(t, work, kv=kv)
            drain(work)
            if t < NQT - 1:
                pend = normgather_gen(t, pz, zb[t], zg[t])
                ogens[t] = oproj_band_gen(t, zg[t], 0, TQ)
            else:
                drain(normgather_gen(t, pz, zb[t], zg[t], last=True))
                drain(oproj_band_gen(t, zg[t], 0, TQ))

    nc.compile()
    return nc


def _chain(gens):
    for g in gens:
        yield from g


def _get_graph() -> bass.Bass:
    if "nc" not in _CACHE:
        _CACHE["nc"] = _build()
    return _CACHE["nc"]


def _make_mask() -> np.ndarray:
    import ml_dtypes

    m = np.empty((KC, G * TQ), np.float32)
    x = np.arange(KC)[:, None]
    y = np.arange(TQ)[None, :]
    for dc in range(G):
        m[:, dc * TQ : (dc + 1) * TQ] = (dc * KC + x <= y).astype(np.float32)
    return m.astype(ml_dtypes.bfloat16)


def _make_in_maps(inputs: dict) -> list[dict]:
    import ml_dtypes

    bf16 = ml_dtypes.bfloat16
    qx = np.asarray(inputs["query_input"], np.float32).astype(bf16)
    kx = np.asarray(inputs["key_input"], np.float32).astype(bf16)
    vx = np.asarray(inputs["value_input"], np.float32).astype(bf16)
    # fold the attention scale into W_Q on the host
    WQ = (np.asarray(inputs["W_Q"], np.float32) / SCALE).astype(bf16)
    WK = np.asarray(inputs["W_K"], np.float32).astype(bf16)
    WV = np.asarray(inputs["W_V"], np.float32).astype(bf16)
    WO = np.asarray(inputs["W_O"], np.float32).astype(bf16)

    mask = _make_mask()
    xT = {
        (nm, b): np.ascontiguousarray(arr[b].T)
        for nm, arr in (("xq", qx), ("xk", kx), ("xv", vx))
        for b in range(B)
    }
    WO_flat = WO.reshape(H * DH, D)  # e' = h*64 + e, h-major (AllGather order)
    wmaps = []
    for g in range(G):
        hs = slice(g * G, (g + 1) * G)

        def prep(w):
            return np.ascontiguousarray(
                w[hs].transpose(1, 0, 2).reshape(D, EG).reshape(NDC, DC, EG)
            )

        wmaps.append(
            {
                "wq": prep(WQ),
                "wk": prep(WK),
                "wv": prep(WV),
                "wo": np.ascontiguousarray(
                    WO_flat[:, g * DS : (g + 1) * DS].reshape(NDC, DC, DS)
                ),
            }
        )

    in_maps = []
    for core in range(NCORES):
        b, g = core // G, core % G
        m = {
            "xq": xT[("xq", b)],
            "xk": xT[("xk", b)],
            "xv": xT[("xv", b)],
            "mask": mask,
        }
        m.update(wmaps[g])
        in_maps.append(m)
    return in_maps


def _assemble(results: list[dict]) -> np.ndarray:
    out = np.empty((B, S, D), np.float32)
    for core in range(NCORES):
        b, g = core // G, core % G
        out[b, :, g * DS : (g + 1) * DS] = results[core]["out"]
    return out


def run(inputs: dict, trace: bool = False):
    """Run on hardware; returns (output, BassKernelResults)."""
    nc = _get_graph()
    res = run_bass_kernel_spmd(
        nc, _make_in_maps(inputs), core_ids=list(range(NCORES)), trace=trace
    )
    return _assemble(res.results), res


def kernel(**inputs) -> np.ndarray:
    out, _ = run(inputs)
    return out
